# revision 1
# baseline (speedup 1.0000x reference)
"""Trainium2 Bass kernel for a 4-bit-quantized linear layer.

Computes y = x @ W^T + bias where W [O, I] is dequantized on-chip from
packed int4 nibbles with a per-group (16 along I) abs-max scale:
    W[o, i] = (q/15) * 2*norm - norm = (2*norm/15) * (q - 7.5)

Distribution: tensor-parallel over output features. Each of the 8 cores
owns O/8 = 2048 output rows (its slice of quantized_weights /
weight_normalization / bias), the input is replicated, and the host
concatenates the per-core [T, 2048] outputs along the feature axis.

Per-core device program:
  1. Dequantize the weight shard to fp16 in [o, i] layout on DVE
     (bitwise nibble extraction + fused (q - 7.5) * a with a = 2*norm/15),
     then one xbar DMA-transpose per 128-row o-tile into a resident
     [i, o] fp16 tensor in SBUF (3D out AP folds the k-tile dim into the
     partition dim, so the per-op HWDGE fixed cost is paid 16x, not 512x).
  2. Stream x in 128-token blocks: fp32->fp16 cast during the (SWDGE) DMA
     load, one xbar DMA-transpose to all 32 x^T k-tiles, and accumulate
     psum[t, o] += xT_k^T @ WT_k over the 32 k-tiles on the TensorEngine
     (x^T k-tile stationary, 4 psum banks of 512 output features each).
  3. Add bias (DVE scalar_tensor_tensor fused with the PSUM->SBUF copy)
     and DMA out.
"""

import numpy as np

import concourse.bass as bass
import concourse.mybir as mybir
from concourse import bacc
from concourse.tile import TileContext

# Full problem shapes (hardcoded; kernel.py must be self-contained).
B, S = 4, 2048
IN_F = 4096
OUT_F = 16384
GROUP = 16
N_CORES = 8
T_FULL = B * S                    # 8192 tokens
O_SH = OUT_F // N_CORES           # 2048 output features per core
G_SH = O_SH * IN_F // GROUP       # 524288 quant groups per core

F16 = mybir.dt.float16
F32 = mybir.dt.float32
I32 = mybir.dt.int32

P = 128  # partitions


def emit_linear4bit(tc, x_ap, qw_ap, wn_ap, bias_ap, y_ap, T, I, O):
    """Emit the per-core program into TileContext tc.

    x:  [T, I]  f32   (replicated input)
    qw: [O*I/16, 8] i32  (this core's group rows; each i32 holds one byte
                          = two nibbles)
    wn: [O*I/16, 1] f32  (per-group scale)
    bias: [O] f32
    y:  [T, O] f32
    """
    nc = tc.nc
    op = mybir.AluOpType

    KT = I // P                   # k tiles (contraction)
    OT = O // P                   # o tiles for dequant
    OC = min(512, O)              # psum chunk along o
    NOC = O // OC
    TB = T // P                   # token blocks
    GPI = I // GROUP              # groups per output row
    BPG = GROUP // 2              # bytes per group (8)

    # ---- resident tensors -------------------------------------------------
    # W^T, fp16, [i (partition within k-tile), k-tile, o] -> 2*KT*O bytes/part
    singles = tc.alloc_tile_pool(name="singles", bufs=1)
    wT = singles.tile([P, KT, O], F16, tag="wT", name="wT")
    bias_rep = singles.tile([P, O], F32, tag="bias_rep", name="bias_rep")

    # bias replicated across partitions via a broadcast-source DMA
    nc.sync.dma_start(bias_rep[:, :], bias_ap[None, :].broadcast_to([P, O]))

    # views of the weight inputs grouped by 128-row o tiles
    # qw rows: g = o * GPI + gi ; o = ot*128 + p
    qw_r = qw_ap.rearrange("(ot p g) b -> ot p (g b)", ot=OT, p=P, g=GPI)
    wn_r = wn_ap.rearrange("(ot p g) one -> ot p (g one)", ot=OT, p=P, g=GPI)

    # ---- phase 1: dequantize + transpose W --------------------------------
    with (
        tc.tile_pool(name="qpool", bufs=2) as qpool,
        tc.tile_pool(name="spool", bufs=2) as spool,
        tc.tile_pool(name="nibpool", bufs=2) as nibpool,
        tc.tile_pool(name="wdpool", bufs=2) as wdpool,
    ):
        for ot in range(OT):
            qt = qpool.tile([P, GPI * BPG], I32, tag="qt")
            nc.sync.dma_start(qt[:, :], qw_r[ot])
            wnt = spool.tile([P, GPI], F32, tag="wnt")
            nc.sync.dma_start(wnt[:, :], wn_r[ot])
            at = spool.tile([P, GPI], F32, tag="at")
            nc.vector.tensor_scalar_mul(at[:, :], wnt[:, :], 2.0 / 15.0)

            lo = nibpool.tile([P, GPI * BPG], I32, tag="nib")
            hi = nibpool.tile([P, GPI * BPG], I32, tag="nib")
            nc.vector.tensor_scalar(lo[:, :], qt[:, :], 15, None, op0=op.bitwise_and)
            nc.vector.tensor_scalar(
                hi[:, :], qt[:, :], 4, None, op0=op.logical_shift_right
            )

            wd = wdpool.tile([P, I], F16, tag="wd")
            wd4 = wd[:, :].rearrange("p (g b t) -> p g b t", g=GPI, b=BPG, t=2)
            a_b = at[:, :, None].broadcast_to([P, GPI, BPG])
            lo_r = lo[:, :].rearrange("p (g b) -> p g b", b=BPG)
            hi_r = hi[:, :].rearrange("p (g b) -> p g b", b=BPG)
            # W = (q - 7.5) * (2*norm/15)
            nc.vector.scalar_tensor_tensor(
                wd4[:, :, :, 0], lo_r, -7.5, a_b, op0=op.add, op1=op.mult
            )
            nc.vector.scalar_tensor_tensor(
                wd4[:, :, :, 1], hi_r, -7.5, a_b, op0=op.add, op1=op.mult
            )

            # one xbar-transpose for the whole o-tile: logical [I, 128] <-
            # [128, I]; out extra dim k folds into the partition dim
            nc.sync.dma_start_transpose(
                wT[:, :, ot * P : (ot + 1) * P], wd[:, :]
            )

    # ---- phase 2: main matmul loop over token blocks ----------------------
    with (
        tc.tile_pool(name="xfpool", bufs=3) as xfpool,
        tc.tile_pool(name="xTpool", bufs=3) as xTpool,
        tc.tile_pool(name="opool", bufs=8) as opool,
        tc.tile_pool(name="pspool", bufs=8, space="PSUM") as pspool,
    ):
        for tb in range(TB):
            trow = slice(tb * P, (tb + 1) * P)
            # fp32 -> fp16 cast during the DMA (SWDGE)
            xf = xfpool.tile([P, I], F16, tag="xf")
            nc.gpsimd.dma_start(xf[:, :], x_ap[trow, :])
            xT = xTpool.tile([P, KT, P], F16, tag="xT")
            nc.sync.dma_start_transpose(xT[:, :, :], xf[:, :])

            ps = []
            for oc in range(NOC):
                pst = pspool.tile([P, OC], F32, tag="ps")
                ps.append(pst)
            for k in range(KT):
                lhs = xT[:, k, :]
                for oc in range(NOC):
                    nc.tensor.matmul(
                        ps[oc][:, :],
                        lhs,
                        wT[:, k, oc * OC : (oc + 1) * OC],
                        start=(k == 0),
                        stop=(k == KT - 1),
                    )
            for oc in range(NOC):
                osb = opool.tile([P, OC], F32, tag="osb")
                nc.vector.scalar_tensor_tensor(
                    osb[:, :],
                    ps[oc][:, :],
                    0.0,
                    bias_rep[:, oc * OC : (oc + 1) * OC],
                    op0=op.add,
                    op1=op.add,
                )
                nc.sync.dma_start(y_ap[trow, oc * OC : (oc + 1) * OC], osb[:, :])

    singles.release()


def build_nc(T=T_FULL, I=IN_F, O=O_SH):
    nc = bacc.Bacc("TRN2", target_bir_lowering=False, debug=False)
    x = nc.dram_tensor("x", [T, I], F32, kind="ExternalInput")
    qw = nc.dram_tensor("qw", [O * I // GROUP, GROUP // 2], I32, kind="ExternalInput")
    wn = nc.dram_tensor("wn", [O * I // GROUP, 1], F32, kind="ExternalInput")
    b = nc.dram_tensor("bias", [O], F32, kind="ExternalInput")
    y = nc.dram_tensor("y", [T, O], F32, kind="ExternalOutput")
    with TileContext(nc) as tc:
        emit_linear4bit(tc, x.ap(), qw.ap(), wn.ap(), b.ap(), y.ap(), T, I, O)
    nc.compile()
    return nc


TRACE = False
LAST_RESULT = None


def kernel(input_tensor, quantized_weights, weight_normalization, bias):
    global LAST_RESULT
    from concourse.bass_utils import run_bass_kernel_spmd

    x = np.ascontiguousarray(
        np.asarray(input_tensor, dtype=np.float32).reshape(T_FULL, IN_F)
    )
    qw = np.asarray(quantized_weights, dtype=np.int32)
    wn = np.asarray(weight_normalization, dtype=np.float32)
    b = np.asarray(bias, dtype=np.float32)

    nc = build_nc()
    in_maps = []
    for c in range(N_CORES):
        in_maps.append(
            {
                "x": x,
                "qw": np.ascontiguousarray(qw[c * G_SH : (c + 1) * G_SH]),
                "wn": np.ascontiguousarray(wn[c * G_SH : (c + 1) * G_SH]),
                "bias": np.ascontiguousarray(b[c * O_SH : (c + 1) * O_SH]),
            }
        )
    res = run_bass_kernel_spmd(nc, in_maps, list(range(N_CORES)), trace=TRACE)
    LAST_RESULT = res
    y = np.concatenate([r["y"] for r in res.results], axis=1)
    return np.ascontiguousarray(y.reshape(B, S, OUT_F), dtype=np.float32)



# revision 8
# speedup vs baseline: 1.2743x; 1.2743x over previous
"""Trainium2 Bass kernel for a 4-bit-quantized linear layer.

Computes y = x @ W^T + bias where W [O, I] is dequantized on-chip from
packed int4 nibbles with a per-group (16 along I) abs-max scale:
    W[o, i] = (q/15) * 2*norm - norm = (2*norm/15) * (q - 7.5)

Distribution: tensor-parallel over output features. Each of the 8 cores
owns O/8 = 2048 output rows (its slice of quantized_weights /
weight_normalization / bias), the input is replicated, and the host
concatenates the per-core [T, 2048] outputs along the feature axis.

Per-core device program (fp8 DoubleRow formulation):
  The TensorEngine runs fp8e4 matmuls in DoubleRow perf mode (two
  128-deep k-subtiles per instruction). e4m3 alone is too coarse
  (~3.7% rel err), so both operands are split hi/lo and the product
  expanded to three terms:
      y ~= xh@Wh + xh@Wl + xl@Wh        (~0.2% rel err)
  where vh = e4m3(v), vl = e4m3(v - vh), all accumulated into one PSUM
  group over a 3*I-long stacked contraction (3/8 the fp16 cycle count).

  Phase 1 (weights): per 128-row o-tile, DVE extracts nibbles and emits
  the dequantized f16 tile, one xbar DMA-transpose flips it to [i, o],
  the Act engine casts to fp8 Wh and the Pool engine computes the fp8
  residual Wl. W lives as one tile pair per 512-wide psum chunk.

  Phase 2 (tokens): x is pre-cast to f16 in DRAM once (SWDGE), then each
  128-token block is xbar-transposed straight from DRAM, split into
  xh/xl fp8 (Act cast + DVE subtract), and 48 DoubleRow matmuls per
  psum chunk accumulate y^T chunks, finished by a DVE bias-add fused
  with the PSUM->SBUF copy and a DMA store.

  The first two blocks are emitted interleaved with phase 1, walking
  the four W chunks in production order (and the Wl-dependent middle
  segment last), so the TensorEngine starts ~45us into the program
  instead of waiting ~200us for all weights.
"""

import numpy as np

import concourse.bass as bass
import concourse.mybir as mybir
from concourse import bacc
from concourse.tile import TileContext

# Full problem shapes (hardcoded; kernel.py must be self-contained).
B, S = 4, 2048
IN_F = 4096
OUT_F = 16384
GROUP = 16
N_CORES = 8
T_FULL = B * S                    # 8192 tokens
O_SH = OUT_F // N_CORES           # 2048 output features per core
G_SH = O_SH * IN_F // GROUP       # 524288 quant groups per core

F16 = mybir.dt.float16
F32 = mybir.dt.float32
F8 = mybir.dt.float8e4
I32 = mybir.dt.int32
I16 = mybir.dt.int16

P = 128  # partitions


def emit_linear4bit(tc, x_ap, qw_ap, wn_ap, bias_ap, y_ap, T, I, O):
    nc = tc.nc
    op = mybir.AluOpType
    DR = mybir.MatmulPerfMode.DoubleRow

    KT = I // P                   # k subtiles (contraction), 32
    OT = O // P                   # o tiles for dequant, 16
    OC = min(512, O)              # psum chunk along o
    NOC = O // OC                 # 4
    TB = T // P                   # token blocks, 64
    GPI = I // GROUP              # groups per output row, 256
    BPG = GROUP // 2              # bytes per group (8)
    KS2 = KT // 2                 # DoubleRow k-steps per segment, 16
    TPC = OC // P                 # o-tiles per psum chunk, 4
    NWARM = 2                     # blocks interleaved with phase 1

    # ---- pools ------------------------------------------------------------
    singles = tc.alloc_tile_pool(name="singles", bufs=1)
    wh = [singles.tile([P, KT, OC], F8, tag=f"wh{c}", name=f"wh{c}") for c in range(NOC)]
    wl = [singles.tile([P, KT, OC], F8, tag=f"wl{c}", name=f"wl{c}") for c in range(NOC)]
    bias16 = singles.tile([P, O], F16, tag="bias16", name="bias16")

    dram = tc.alloc_tile_pool(name="dram", bufs=1, space="DRAM")
    x16d = dram.tile([TB, P, I], F16, tag="x16d", name="x16d")

    stage = tc.alloc_tile_pool(name="stage", bufs=2)   # wd f16 (+ bias ld)
    stT = tc.alloc_tile_pool(name="stT", bufs=2)       # transposed f16: wt/xT
    qpool = tc.alloc_tile_pool(name="qpool", bufs=1)
    spool = tc.alloc_tile_pool(name="spool", bufs=1)
    nibp = tc.alloc_tile_pool(name="nibp", bufs=1)
    x8p = tc.alloc_tile_pool(name="x8p", bufs=3)
    opool = tc.alloc_tile_pool(name="opool", bufs=2)
    psp = tc.alloc_tile_pool(name="psp", bufs=8, space="PSUM")

    qw_r = qw_ap.rearrange("(ot p g) b -> ot p (g b)", ot=OT, p=P, g=GPI)
    wn_r = wn_ap.rearrange("(ot p g) one -> ot p (g one)", ot=OT, p=P, g=GPI)
    x_r = x_ap.rearrange("(tb p) i -> tb p i", p=P)

    # ---- bias: borrow a stage slot as the f32 landing buffer --------------
    bld = stage.tile([P, I], F16, tag="st", name="st")
    bld32 = bld[:, :].bitcast(F32)  # [P, 2048] f32 view
    nc.sync.dma_start(bld32, bias_ap[None, :].broadcast_to([P, O]))
    nc.scalar.copy(bias16[:, :], bld32)

    # ---- x pre-cast f32 -> f16 into DRAM (SWDGE), 8-block chunks ----------
    for ch in range(TB // 8):
        rows = slice(ch * 8, (ch + 1) * 8)
        nc.gpsimd.dma_start(x16d[rows], x_r[rows])

    def phase1_tile(ot):
        c, osub = divmod(ot, TPC)
        osl = slice(osub * P, (osub + 1) * P)
        qt = qpool.tile([P, GPI * BPG], I32, tag="qt", name="qt")
        nc.sync.dma_start(qt[:, :], qw_r[ot])
        wnt = spool.tile([P, GPI], F32, tag="wnt", name="wnt")
        nc.sync.dma_start(wnt[:, :], wn_r[ot])
        at = spool.tile([P, GPI], F32, tag="at", name="at")
        nc.scalar.mul(at[:, :], wnt[:, :], 2.0 / 15.0)

        # hi nibble first (i16 out), then mask qt in place for the low one
        hi = nibp.tile([P, GPI * BPG], I16, tag="hi", name="hi")
        nc.vector.tensor_scalar(
            hi[:, :], qt[:, :], 4, None, op0=op.logical_shift_right
        )
        nc.vector.tensor_scalar(qt[:, :], qt[:, :], 15, None, op0=op.bitwise_and)

        wd = stage.tile([P, I], F16, tag="st", name="st")
        wd4 = wd[:, :].rearrange("p (g b t) -> p g b t", g=GPI, b=BPG, t=2)
        a_b = at[:, :, None].broadcast_to([P, GPI, BPG])
        lo_r = qt[:, :].rearrange("p (g b) -> p g b", b=BPG)
        hi_r = hi[:, :].rearrange("p (g b) -> p g b", b=BPG)
        # W = (q - 7.5) * (2*norm/15)
        nc.vector.scalar_tensor_tensor(
            wd4[:, :, :, 0], lo_r, -7.5, a_b, op0=op.add, op1=op.mult
        )
        nc.vector.scalar_tensor_tensor(
            wd4[:, :, :, 1], hi_r, -7.5, a_b, op0=op.add, op1=op.mult
        )

        wt = stT.tile([P, KT, P], F16, tag="stT", name="stT")
        nc.sync.dma_start_transpose(wt[:, :, :], wd[:, :])
        # fp8 split: cast on Act, residual on Pool
        nc.scalar.copy(wh[c][:, :, osl], wt[:, :, :])
        nc.gpsimd.tensor_tensor(
            wl[c][:, :, osl], wt[:, :, :], wh[c][:, :, osl], op.subtract
        )

    def xprep(b):
        xT = stT.tile([P, KT, P], F16, tag="stT", name="stT")
        nc.sync.dma_start_transpose(xT[:, :, :], x16d[b])
        xh = x8p.tile([P, KT, P], F8, tag="xh", name="xh")
        nc.scalar.copy(xh[:, :, :], xT[:, :, :])
        xl = x8p.tile([P, KT, P], F8, tag="xl", name="xl")
        nc.vector.tensor_sub(xl[:, :, :], xT[:, :, :], xh[:, :, :])
        return xh, xl

    def mm(ps, c, xh, xl, segs, start, stop):
        """DoubleRow matmuls of the listed segments into psum chunk c.

        seg 0: xh@Wh, seg 1: xh@Wl, seg 2: xl@Wh; the caller orders seg 1
        last so only the final third waits on Wl."""
        for j, seg in enumerate(segs):
            lhs8 = xh if seg < 2 else xl
            wsrc = wl[c] if seg == 1 else wh[c]
            for i in range(KS2):
                ksl = slice(2 * i, 2 * i + 2)
                nc.tensor.matmul(
                    ps[:, :],
                    lhs8[:, ksl, :],
                    wsrc[:, ksl, :],
                    start=(start and j == 0 and i == 0),
                    stop=(stop and j == len(segs) - 1 and i == KS2 - 1),
                    perf_mode=DR,
                )

    def finish(b, c, ps):
        trow = slice(b * P, (b + 1) * P)
        osb = opool.tile([P, OC], F32, tag="osb", name="osb")
        nc.vector.scalar_tensor_tensor(
            osb[:, :],
            ps[:, :],
            0.0,
            bias16[:, c * OC : (c + 1) * OC],
            op0=op.add,
            op1=op.add,
        )
        nc.sync.dma_start(y_ap[trow, c * OC : (c + 1) * OC], osb[:, :])

    # ---- warmup: phase 1 interleaved with the first NWARM blocks ----------
    xhl = {}
    for b in range(NWARM):
        xhl[b] = xprep(b)
    for c in range(NOC):
        for ot in range(c * TPC, (c + 1) * TPC):
            phase1_tile(ot)
        psc = [psp.tile([P, OC], F32, tag="ps", name="ps") for _ in range(NWARM)]
        for b in range(NWARM):
            mm(psc[b], c, *xhl[b], segs=[0, 2], start=True, stop=False)
        for b in range(NWARM):
            mm(psc[b], c, *xhl[b], segs=[1], start=False, stop=True)
            finish(b, c, psc[b])
        if c == 0:
            xhl[NWARM] = xprep(NWARM)

    # ---- steady state -----------------------------------------------------
    for b in range(NWARM, TB):
        if b + 1 < TB:
            xhl[b + 1] = xprep(b + 1)
        xh, xl = xhl.pop(b)
        ps = [psp.tile([P, OC], F32, tag="ps", name="ps") for _ in range(NOC)]
        for seg in (0, 2, 1):
            lhs8 = xh if seg < 2 else xl
            for i in range(KS2):
                ksl = slice(2 * i, 2 * i + 2)
                for c in range(NOC):
                    wsrc = wl[c] if seg == 1 else wh[c]
                    nc.tensor.matmul(
                        ps[c][:, :],
                        lhs8[:, ksl, :],
                        wsrc[:, ksl, :],
                        start=(seg == 0 and i == 0),
                        stop=(seg == 1 and i == KS2 - 1),
                        perf_mode=DR,
                    )
        for c in range(NOC):
            finish(b, c, ps[c])

    psp.release()
    opool.release()
    x8p.release()
    nibp.release()
    spool.release()
    qpool.release()
    stT.release()
    stage.release()
    dram.release()
    singles.release()


def build_nc(T=T_FULL, I=IN_F, O=O_SH):
    nc = bacc.Bacc("TRN2", target_bir_lowering=False, debug=False)
    x = nc.dram_tensor("x", [T, I], F32, kind="ExternalInput")
    qw = nc.dram_tensor("qw", [O * I // GROUP, GROUP // 2], I32, kind="ExternalInput")
    wn = nc.dram_tensor("wn", [O * I // GROUP, 1], F32, kind="ExternalInput")
    b = nc.dram_tensor("bias", [O], F32, kind="ExternalInput")
    y = nc.dram_tensor("y", [T, O], F32, kind="ExternalOutput")
    with TileContext(nc) as tc:
        emit_linear4bit(tc, x.ap(), qw.ap(), wn.ap(), b.ap(), y.ap(), T, I, O)
    nc.compile()
    return nc


TRACE = False
LAST_RESULT = None


def kernel(input_tensor, quantized_weights, weight_normalization, bias):
    global LAST_RESULT
    from concourse.bass_utils import run_bass_kernel_spmd

    x = np.ascontiguousarray(
        np.asarray(input_tensor, dtype=np.float32).reshape(T_FULL, IN_F)
    )
    qw = np.asarray(quantized_weights, dtype=np.int32)
    wn = np.asarray(weight_normalization, dtype=np.float32)
    b = np.asarray(bias, dtype=np.float32)

    nc = build_nc()
    in_maps = []
    for c in range(N_CORES):
        in_maps.append(
            {
                "x": x,
                "qw": np.ascontiguousarray(qw[c * G_SH : (c + 1) * G_SH]),
                "wn": np.ascontiguousarray(wn[c * G_SH : (c + 1) * G_SH]),
                "bias": np.ascontiguousarray(b[c * O_SH : (c + 1) * O_SH]),
            }
        )
    res = run_bass_kernel_spmd(nc, in_maps, list(range(N_CORES)), trace=TRACE)
    LAST_RESULT = res
    y = np.concatenate([r["y"] for r in res.results], axis=1)
    return np.ascontiguousarray(y.reshape(B, S, OUT_F), dtype=np.float32)
